# revision 8
# baseline (speedup 1.0000x reference)
"""AVRNN (VRNN + 2-layer dense GCN) forward on 8 Trainium2 NeuronCores.

Strategy:
  - Batch (agent) dim B=1024 sharded 8 ways: core j owns rows j*128:(j+1)*128.
  - All MLP/VAE/GRU compute is row-local in a feature-major layout
    (features on SBUF partitions, rows on the free dim); matmuls are
    bf16 x bf16 -> fp32 PSUM.
  - The dense graph conv contracts over the full batch, so twice per step
    the (128,256) per-core matrices (M1 = h@Wg1, N2 = h1@Wg2) are
    exchanged to form the full (1024,256) operand on every core:
      * mode "collective": AllGather via DRAM bounce buffers.
      * mode "remote": direct SBUF->SBUF remote DMA pushes between the 8
        cores; the host pre-permutes each core's adjacency slice (slot d
        holds B-chunk j XOR d for core j) so the SPMD program is identical
        on every core. Consumers gate on data-arrival semaphores; the
        recurrence's own waits transitively protect buffer reuse (double-
        buffered full tiles, max cross-core skew < 1 exchange round).
  - adj is fed pre-transposed + pre-sliced from the host as bf16, so no
    on-device transposes are needed.
  - phi_x (input-side MLP) for all steps is precomputed in a batched phase
    before the recurrence.
  - KLD/NLL are reduced on-device to per-(step,feature) partial sums; the
    final tiny reduction happens on the host.

kernel(**inputs) takes the full unsharded inputs and returns
(KLD[1], NLL[1], h[1,B,RD]) matching reference.reference().
"""

import numpy as np
import ml_dtypes

import concourse.bass as bass
import concourse.mybir as mybir
import concourse.tile as tile
from concourse import bacc
from concourse.bass_utils import run_bass_kernel_spmd

BF = mybir.dt.bfloat16
F32 = mybir.dt.float32
AF = mybir.ActivationFunctionType
ALU = mybir.AluOpType

T, B = 40, 1024
XD, ZD, HD, RD, GH = 2, 64, 256, 256, 256
NSTEPS = T - 1
NCORES = 8
R = B // NCORES          # rows per core = 128
NEG = 0.01               # leaky relu slope
LOG2PI = float(np.log(2.0 * np.pi))

EXCHANGE_MODE = "remote"   # "collective" or "remote"


# ----------------------------------------------------------------- build

def build_kernel(nsteps=NSTEPS, exchange_mode=EXCHANGE_MODE):
    nc = bacc.Bacc(
        "TRN2", target_bir_lowering=False, debug=False,
        enable_asserts=True, num_devices=NCORES,
    )

    S = nsteps * R  # total row-steps in the x/eps strips

    # ---- DRAM inputs (per core) ----
    d_xbf = nc.dram_tensor("xbf", [XD, S], BF, kind="ExternalInput")
    d_xf = nc.dram_tensor("xf", [XD, S], F32, kind="ExternalInput")
    d_eps = nc.dram_tensor("eps", [ZD, S], F32, kind="ExternalInput")
    d_adjt = nc.dram_tensor("adjt", [nsteps, 128, NCORES, R], BF,
                            kind="ExternalInput")

    wspec = {
        'Wpx1': (XD, HD), 'Wpx2': (HD, HD),
        'Wpz1': (ZD, HD), 'Wpz2': (HD, HD),
        'We1': (HD + RD, HD), 'We2': (HD, HD),
        'Wmlv_e': (HD, 2 * ZD),
        'Wp1': (RD, HD), 'Wp2': (HD, HD),
        'Wmlv_p': (HD, 2 * ZD),
        'Wd1': (HD + RD, HD), 'Wd2': (HD, HD),
        'Wmlv_d': (HD, 32 + XD),
        'Wih': (2 * HD, 3 * RD), 'Whh': (RD, 3 * RD),
        'Wg1': (RD, GH), 'Wg2': (GH, RD),
        'Wlg': (2 * RD, RD),
    }
    bspec = {
        'bpx1': HD, 'bpx2': HD, 'bpz1': HD, 'bpz2': HD,
        'be1': HD, 'be2': HD, 'bmlv_e': 2 * ZD,
        'bp1': HD, 'bp2': HD, 'bmlv_p': 2 * ZD,
        'bd1': HD, 'bd2': HD, 'bmlv_d': 32 + XD,
        'brz': 2 * RD,          # 0.5*(bih+bhh) for r,z gates
        'bihn': RD, 'bhhn': RD,
        'bg1': GH, 'bg2': RD, 'blg': RD,
    }
    d_w = {k: nc.dram_tensor(k, list(v), BF, kind="ExternalInput")
           for k, v in wspec.items()}
    d_b = {k: nc.dram_tensor(k, [v], F32, kind="ExternalInput")
           for k, v in bspec.items()}

    # ---- DRAM outputs (per core) ----
    d_hout = nc.dram_tensor("hout", [128, 2, R], F32, kind="ExternalOutput")
    d_kld = nc.dram_tensor("kldacc", [ZD, 3 * nsteps], F32, kind="ExternalOutput")
    d_nll = nc.dram_tensor("nllacc", [XD, 2 * nsteps], F32, kind="ExternalOutput")

    remote = exchange_mode == "remote"
    if remote:
        sem_dm = nc.alloc_semaphore("sem_dm")   # M1 data arrivals
        sem_dn = nc.alloc_semaphore("sem_dn")   # N2 data arrivals
        sem_lm = nc.alloc_semaphore("sem_lm")   # M1 local send-complete
        sem_ln = nc.alloc_semaphore("sem_ln")   # N2 local send-complete

    rg = [list(range(NCORES))]

    with tile.TileContext(nc) as tc:
        with (
            tc.tile_pool(name="const", bufs=1) as const,
            tc.tile_pool(name="work", bufs=3) as work,
            tc.tile_pool(name="state", bufs=2) as state,
            tc.tile_pool(name="adjp", bufs=3) as adjp,
            tc.tile_pool(name="psum", bufs=2, space="PSUM") as psum,
            tc.tile_pool(name="dram", bufs=2, space="DRAM") as dram,
        ):
            # ---- startup barrier + semaphore clear (remote mode) ----
            if remote:
                bar_sb = const.tile([128, 16], BF, tag="bar_sb")
                with tc.tile_critical():
                    nc.gpsimd.sem_clear(sem_dm)
                    nc.gpsimd.sem_clear(sem_dn)
                    nc.gpsimd.sem_clear(sem_lm)
                    nc.gpsimd.sem_clear(sem_ln)
                    nc.gpsimd.memset(bar_sb[:], 0.0)
                bar_in = dram.tile([128, 16], BF, tag="bar_in")
                bar_out = dram.tile([NCORES * 128, 16], BF, tag="bar_out")
                nc.sync.dma_start(bar_in[:], bar_sb[:])
                nc.gpsimd.collective_compute(
                    "AllGather", ALU.bypass,
                    ins=[bar_in.opt()], outs=[bar_out.opt()],
                    replica_groups=rg)
                bar_back = const.tile([128, 16], BF, tag="bar_back")
                nc.sync.dma_start(bar_back[:], bar_out[:].rearrange(
                    "(c p) n -> p c n", p=128)[:, 0, :])

            # ---- load weights and biases into SBUF ----
            w = {}
            for k, (K, N) in wspec.items():
                kc = (K + 127) // 128
                if K >= 128:
                    t_ = const.tile([128, kc, N], BF, tag=f"w_{k}")
                    nc.sync.dma_start(
                        t_[:], d_w[k][:].rearrange("(kc p) n -> p kc n", p=128))
                else:
                    t_ = const.tile([K, 1, N], BF, tag=f"w_{k}")
                    nc.sync.dma_start(t_[:], d_w[k][:].rearrange("p (o n) -> p o n", o=1))
                w[k] = t_
            bia = {}
            for k, n in bspec.items():
                if n >= 128:
                    ch = n // 128
                    t_ = const.tile([128, ch], F32, tag=f"b_{k}")
                    nc.sync.dma_start(
                        t_[:], d_b[k][:].rearrange("(c p) -> p c", p=128))
                else:
                    t_ = const.tile([n, 1], F32, tag=f"b_{k}")
                    nc.sync.dma_start(t_[:], d_b[k][:].rearrange("(p o) -> p o", o=1))
                bia[k] = t_

            def bsl(name, c):  # bias (P,1) slice for chunk c
                return bia[name][:, c:c + 1]

            # negated biases for the ACT-path lrelu layers
            negb = {}
            for k in ('be1', 'be2', 'bp1', 'bp2'):
                nb = const.tile([128, 2], F32, tag=f"nb_{k}")
                nc.vector.tensor_scalar_mul(nb[:], bia[k][:], -1.0)
                negb[k] = nb

            def nbsl(name, c):
                return negb[name][:, c:c + 1]

            # x / eps strips
            x_bf = const.tile([XD, S], BF, tag="x_bf")
            nc.sync.dma_start(x_bf[:], d_xbf[:])
            x_f = const.tile([XD, S], F32, tag="x_f")
            nc.sync.dma_start(x_f[:], d_xf[:])
            eps_f = const.tile([ZD, S], F32, tag="eps_f")
            nc.sync.dma_start(eps_f[:], d_eps[:])

            # accumulator strips
            kldacc = const.tile([ZD, 3 * nsteps], F32, tag="kldacc")
            nllacc = const.tile([XD, 2 * nsteps], F32, tag="nllacc")

            # h state (bf16 feature-major: [feat_in_chunk, chunk*R + row])
            h_prev = state.tile([128, 2 * R], BF, tag="h")
            nc.vector.memset(h_prev[:], 0.0)

            # exchange target buffers: fixed addresses (required for remote)
            mfull = [const.tile([128, NCORES, 2 * R], BF, tag=f"mfull{p}",
                                name=f"mfull{p}") for p in range(2)]
            nfull = [const.tile([128, NCORES, 2 * R], BF, tag=f"nfull{p}",
                                name=f"nfull{p}") for p in range(2)]

            # ---------------- helpers ----------------
            def lrelu_evac_dve(ps, bname, bc, out_ap):
                """out = lrelu(psum + bias) via 2 DVE ops."""
                t1 = work.tile([128, R], BF, tag="lr_t1")
                nc.vector.tensor_scalar(
                    t1[:], ps, bsl(bname, bc), NEG, ALU.add, ALU.mult)
                nc.vector.scalar_tensor_tensor(
                    out_ap, ps, bsl(bname, bc), t1[:], ALU.add, ALU.max)

            def lrelu_evac_act(ps, bname, bc, out_ap):
                """out = lrelu(psum+b) via 2 ACT relus + 1 DVE combine."""
                p_ = work.tile([128, R], BF, tag="lr_p")
                m_ = work.tile([128, R], BF, tag="lr_m")
                nc.scalar.activation(p_[:], ps, AF.Relu, bias=bsl(bname, bc))
                nc.scalar.activation(m_[:], ps, AF.Relu, bias=nbsl(bname, bc),
                                     scale=-1.0)
                nc.vector.scalar_tensor_tensor(
                    out_ap, m_[:], -NEG, p_[:], ALU.mult, ALU.add)

            def mlp2_fm(rhs_slices, w1, b1, w2, b2, out_tile, mode):
                """Two leaky-relu layers, feature-major in/out (128, 2R)."""
                kc1 = len(rhs_slices)
                ps1 = psum.tile([128, 2 * R], F32, tag="ps1")
                for m in range(2):
                    for k in range(kc1):
                        nc.tensor.matmul(
                            ps1[:, m * R:(m + 1) * R],
                            w1[:, k, m * 128:(m + 1) * 128],
                            rhs_slices[k],
                            start=(k == 0), stop=(k == kc1 - 1))
                mid = work.tile([128, 2 * R], BF, tag="mlp_mid")
                for c in range(2):
                    sl = ps1[:, c * R:(c + 1) * R]
                    out = mid[:, c * R:(c + 1) * R]
                    if mode == 'act':
                        lrelu_evac_act(sl, b1, c, out)
                    else:
                        lrelu_evac_dve(sl, b1, c, out)
                ps2 = psum.tile([128, 2 * R], F32, tag="ps2")
                for m in range(2):
                    for k in range(2):
                        nc.tensor.matmul(
                            ps2[:, m * R:(m + 1) * R],
                            w2[:, k, m * 128:(m + 1) * 128],
                            mid[:, k * R:(k + 1) * R],
                            start=(k == 0), stop=(k == 1))
                for c in range(2):
                    sl = ps2[:, c * R:(c + 1) * R]
                    out = out_tile[:, c * R:(c + 1) * R]
                    if mode == 'act':
                        lrelu_evac_act(sl, b2, c, out)
                    else:
                        lrelu_evac_dve(sl, b2, c, out)

            # ---------------- phase A: phi_x for all steps ----------------
            phix = const.tile([128, 2, S], BF, tag="phix")
            off = 0
            while off < S:
                L = min(512, S - off)
                mids = work.tile([128, 2, 512], BF, tag="px_mid")
                for m in range(2):
                    psx = psum.tile([128, 512], F32, tag="ps1")
                    nc.tensor.matmul(
                        psx[:, :L], w['Wpx1'][:, 0, m * 128:(m + 1) * 128],
                        x_bf[:, off:off + L], start=True, stop=True)
                    t1 = work.tile([128, 512], BF, tag="px_t1")
                    nc.vector.tensor_scalar(
                        t1[:, :L], psx[:, :L], bsl('bpx1', m), NEG,
                        ALU.add, ALU.mult)
                    nc.vector.scalar_tensor_tensor(
                        mids[:, m, :L], psx[:, :L], bsl('bpx1', m), t1[:, :L],
                        ALU.add, ALU.max)
                for m in range(2):
                    psx = psum.tile([128, 512], F32, tag="ps2")
                    for k in range(2):
                        nc.tensor.matmul(
                            psx[:, :L], w['Wpx2'][:, k, m * 128:(m + 1) * 128],
                            mids[:, k, :L], start=(k == 0), stop=(k == 1))
                    t1 = work.tile([128, 512], BF, tag="px_t1")
                    nc.vector.tensor_scalar(
                        t1[:, :L], psx[:, :L], bsl('bpx2', m), NEG,
                        ALU.add, ALU.mult)
                    nc.vector.scalar_tensor_tensor(
                        phix[:, m, off:off + L], psx[:, :L], bsl('bpx2', m),
                        t1[:, :L], ALU.add, ALU.max)
                off += L

            # ---------------- exchange implementations ----------------
            def exchange_collective(local_bf, full_tile, tag):
                ag_in = dram.tile([128, 2 * R], BF, tag=f"agi_{tag}")
                ag_out = dram.tile([NCORES * 128, 2 * R], BF, tag=f"ago_{tag}")
                nc.sync.dma_start(ag_in[:], local_bf[:])
                nc.gpsimd.collective_compute(
                    "AllGather", ALU.bypass,
                    ins=[ag_in.opt()], outs=[ag_out.opt()],
                    replica_groups=rg)
                nc.sync.dma_start(
                    full_tile[:], ag_out[:].rearrange("(c p) n -> p c n", p=128))

            rem = {'dm': 0, 'dn': 0}

            def exchange_remote(local_bf, full_tile, data_sem, loc_sem, dkey):
                for d in range(1, NCORES):
                    rdests = [None] * NCORES
                    rdests[d] = (0, d)
                    nc.gpsimd.remote_dma_broadcast(
                        full_tile[:, d, :], local_bf[:],
                        data_sem, loc_sem, rdests=rdests)
                nc.gpsimd.trigger_dma(count=None)
                rem[dkey] += 7 * 2
                # own shard: local copy into slot 0
                nc.vector.tensor_copy(full_tile[:, 0, :], local_bf[:])

            # ---------------- the recurrence ----------------
            for t in range(nsteps):
                xsl = slice(t * R, (t + 1) * R)
                par = t % 2
                m1f = mfull[par]
                n2f = nfull[par]

                # adjacency prefetch (slot layout (p, d, r))
                adjt = adjp.tile([128, NCORES, R], BF, tag="adjt")
                nc.sync.dma_start(adjt[:], d_adjt[t])

                # --- encoder ---
                enc2 = work.tile([128, 2 * R], BF, tag="enc2")
                mlp2_fm([phix[:, 0, xsl], phix[:, 1, xsl],
                         h_prev[:, 0:R], h_prev[:, R:2 * R]],
                        w['We1'], 'be1', w['We2'], 'be2', enc2, 'act')
                ps_me = psum.tile([128, R], F32, tag="ps3")
                for k in range(2):
                    nc.tensor.matmul(ps_me[:], w['Wmlv_e'][:, k, :],
                                     enc2[:, k * R:(k + 1) * R],
                                     start=(k == 0), stop=(k == 1))
                mlv_e = work.tile([128, R], F32, tag="mlv_e")
                nc.vector.tensor_scalar_add(mlv_e[:], ps_me[:],
                                            bia['bmlv_e'][:, 0:1])

                # --- prior ---
                pr2 = work.tile([128, 2 * R], BF, tag="pr2")
                mlp2_fm([h_prev[:, 0:R], h_prev[:, R:2 * R]],
                        w['Wp1'], 'bp1', w['Wp2'], 'bp2', pr2, 'act')
                ps_mp = psum.tile([128, R], F32, tag="ps3")
                for k in range(2):
                    nc.tensor.matmul(ps_mp[:], w['Wmlv_p'][:, k, :],
                                     pr2[:, k * R:(k + 1) * R],
                                     start=(k == 0), stop=(k == 1))
                mlv_p = work.tile([128, R], F32, tag="mlv_p")
                nc.vector.tensor_scalar_add(mlv_p[:], ps_mp[:],
                                            bia['bmlv_p'][:, 0:1])

                # --- z = eps*exp(0.5*elv) + em ---
                sd = work.tile([ZD, R], F32, tag="sd")
                nc.scalar.activation(sd[:], mlv_e[ZD:2 * ZD, :], AF.Exp, scale=0.5)
                zt = work.tile([ZD, R], F32, tag="zt")
                nc.vector.tensor_mul(zt[:], eps_f[:, xsl], sd[:])
                z_bf = work.tile([ZD, R], BF, tag="z_bf")
                nc.vector.tensor_add(z_bf[:], zt[:], mlv_e[0:ZD, :])

                # --- phi_z ---
                phz = work.tile([128, 2 * R], BF, tag="phz")
                mlp2_fm([z_bf[:]], w['Wpz1'], 'bpz1', w['Wpz2'], 'bpz2',
                        phz, 'dve')

                # --- decoder ---
                dec2 = work.tile([128, 2 * R], BF, tag="dec2")
                mlp2_fm([phz[:, 0:R], phz[:, R:2 * R],
                         h_prev[:, 0:R], h_prev[:, R:2 * R]],
                        w['Wd1'], 'bd1', w['Wd2'], 'bd2', dec2, 'dve')
                ps_md = psum.tile([32 + XD, R], F32, tag="ps3")
                for k in range(2):
                    nc.tensor.matmul(ps_md[:], w['Wmlv_d'][:, k, :],
                                     dec2[:, k * R:(k + 1) * R],
                                     start=(k == 0), stop=(k == 1))
                mlv_d = work.tile([32 + XD, R], F32, tag="mlv_d")
                nc.vector.tensor_scalar_add(mlv_d[:], ps_md[:],
                                            bia['bmlv_d'][:, 0:1])

                # --- GRU r,z gates: psum accumulates gi+gh ---
                gi_rhs = [phix[:, 0, xsl], phix[:, 1, xsl],
                          phz[:, 0:R], phz[:, R:2 * R]]
                ps_rz = psum.tile([128, 4 * R], F32, tag="ps4")
                for g in range(4):  # chunks r0 r1 z0 z1
                    sl = ps_rz[:, g * R:(g + 1) * R]
                    for k in range(4):
                        nc.tensor.matmul(sl, w['Wih'][:, k, g * 128:(g + 1) * 128],
                                         gi_rhs[k], start=(k == 0), stop=False)
                    for k in range(2):
                        nc.tensor.matmul(sl, w['Whh'][:, k, g * 128:(g + 1) * 128],
                                         h_prev[:, k * R:(k + 1) * R],
                                         start=False, stop=(k == 1))
                t_rz = work.tile([128, 4 * R], BF, tag="t_rz")
                for g in range(4):
                    nc.scalar.activation(
                        t_rz[:, g * R:(g + 1) * R], ps_rz[:, g * R:(g + 1) * R],
                        AF.Tanh, bias=bsl('brz', g), scale=0.5)
                rz = work.tile([128, 4 * R], BF, tag="rz")
                nc.vector.tensor_scalar(rz[:], t_rz[:], 0.5, 0.5,
                                        ALU.mult, ALU.add)
                r_ap = rz[:, 0:2 * R]
                zg_ap = rz[:, 2 * R:4 * R]

                # --- GRU n gate ---
                ps_gin = psum.tile([128, 2 * R], F32, tag="ps3")
                for g in range(2):
                    sl = ps_gin[:, g * R:(g + 1) * R]
                    for k in range(4):
                        nc.tensor.matmul(
                            sl, w['Wih'][:, k, 512 + g * 128:512 + (g + 1) * 128],
                            gi_rhs[k], start=(k == 0), stop=(k == 3))
                ps_ghn = psum.tile([128, 2 * R], F32, tag="ps4")
                for g in range(2):
                    sl = ps_ghn[:, g * R:(g + 1) * R]
                    for k in range(2):
                        nc.tensor.matmul(
                            sl, w['Whh'][:, k, 512 + g * 128:512 + (g + 1) * 128],
                            h_prev[:, k * R:(k + 1) * R],
                            start=(k == 0), stop=(k == 1))
                G = work.tile([128, 2 * R], BF, tag="G")
                for g in range(2):
                    nc.vector.tensor_scalar_add(
                        G[:, g * R:(g + 1) * R], ps_ghn[:, g * R:(g + 1) * R],
                        bsl('bhhn', g))
                rG = work.tile([128, 2 * R], BF, tag="rG")
                nc.vector.tensor_mul(rG[:], r_ap, G[:])
                npre = work.tile([128, 2 * R], F32, tag="npre")
                for g in range(2):
                    nc.vector.scalar_tensor_tensor(
                        npre[:, g * R:(g + 1) * R], ps_gin[:, g * R:(g + 1) * R],
                        bsl('bihn', g), rG[:, g * R:(g + 1) * R],
                        ALU.add, ALU.add)
                n_t = work.tile([128, 2 * R], BF, tag="n_t")
                nc.scalar.activation(n_t[:], npre[:], AF.Tanh)

                # --- h_new = n + zg*(h-n) ---
                dh = work.tile([128, 2 * R], BF, tag="dh")
                nc.vector.tensor_sub(dh[:], h_prev[:], n_t[:])
                zd = work.tile([128, 2 * R], BF, tag="zd")
                nc.vector.tensor_mul(zd[:], zg_ap, dh[:])
                h_new = work.tile([128, 2 * R], BF, tag="h_new")
                nc.vector.tensor_add(h_new[:], n_t[:], zd[:])

                # --- M1 = h_new @ Wg1 (row-major out) ---
                ps_m1 = psum.tile([128, 2 * R], F32, tag="ps3")
                for k in range(2):
                    nc.tensor.matmul(ps_m1[:], h_new[:, k * R:(k + 1) * R],
                                     w['Wg1'][:, k, :],
                                     start=(k == 0), stop=(k == 1))
                m1_loc = state.tile([128, 2 * R], BF, tag="m1_loc")
                nc.vector.tensor_copy(m1_loc[:], ps_m1[:])

                # --- exchange 1 ---
                if remote:
                    exchange_remote(m1_loc, m1f, sem_dm, sem_lm, 'dm')
                else:
                    exchange_collective(m1_loc, m1f, "m")

                # --- KLD terms (off the critical path) ---
                t1 = work.tile([128, R], F32, tag="kld_t1")
                nc.vector.tensor_sub(t1[:], mlv_p[:], mlv_e[:])
                nc.vector.tensor_reduce(
                    kldacc[:, 3 * t:3 * t + 1], t1[ZD:2 * ZD, :],
                    mybir.AxisListType.X, ALU.add)
                scr = work.tile([ZD, R], BF, tag="kld_scr")
                nc.scalar.activation(scr[:], t1[ZD:2 * ZD, :], AF.Exp,
                                     scale=-1.0,
                                     accum_out=kldacc[:, 3 * t + 1:3 * t + 2])
                e3 = work.tile([ZD, R], F32, tag="kld_e3")
                nc.scalar.activation(e3[:], mlv_p[ZD:2 * ZD, :], AF.Exp,
                                     scale=-0.5)
                u = work.tile([ZD, R], F32, tag="kld_u")
                nc.vector.tensor_mul(u[:], t1[0:ZD, :], e3[:])
                scr2 = work.tile([ZD, R], BF, tag="kld_scr2")
                nc.scalar.activation(scr2[:], u[:], AF.Square,
                                     accum_out=kldacc[:, 3 * t + 2:3 * t + 3])

                # --- NLL terms ---
                en = work.tile([XD, R], F32, tag="nll_e")
                nc.scalar.activation(en[:], mlv_d[32:32 + XD, :], AF.Exp,
                                     scale=-1.0)
                dn_ = work.tile([XD, R], F32, tag="nll_d")
                nc.vector.tensor_sub(dn_[:], x_f[:, xsl], mlv_d[0:XD, :])
                d2 = work.tile([XD, R], F32, tag="nll_d2")
                nc.vector.tensor_mul(d2[:], dn_[:], dn_[:])
                q = work.tile([XD, R], F32, tag="nll_q")
                nc.vector.tensor_mul(q[:], d2[:], en[:])
                nc.vector.tensor_reduce(
                    nllacc[:, 2 * t:2 * t + 1], q[:],
                    mybir.AxisListType.X, ALU.add)
                nc.vector.tensor_reduce(
                    nllacc[:, 2 * t + 1:2 * t + 2], mlv_d[32:32 + XD, :],
                    mybir.AxisListType.X, ALU.add)

                # --- GCN layer 1: h1 = relu(A @ M1 + bg1), feature-major ---
                ps_g1 = psum.tile([128, 2 * R], F32, tag="ps4")
                if remote:
                    with tc.tile_critical():
                        nc.tensor.wait_ge(sem_dm, rem['dm'])
                        for m in range(2):
                            sl = ps_g1[:, m * R:(m + 1) * R]
                            for c in range(NCORES):
                                nc.tensor.matmul(
                                    sl, m1f[:, c, m * 128:(m + 1) * 128],
                                    adjt[:, c, :],
                                    start=(c == 0), stop=(c == NCORES - 1))
                else:
                    for m in range(2):
                        sl = ps_g1[:, m * R:(m + 1) * R]
                        for c in range(NCORES):
                            nc.tensor.matmul(
                                sl, m1f[:, c, m * 128:(m + 1) * 128],
                                adjt[:, c, :],
                                start=(c == 0), stop=(c == NCORES - 1))
                h1 = work.tile([128, 2 * R], BF, tag="h1")
                for c in range(2):
                    nc.vector.tensor_scalar(
                        h1[:, c * R:(c + 1) * R], ps_g1[:, c * R:(c + 1) * R],
                        bsl('bg1', c), 0.0, ALU.add, ALU.max)

                # --- N2 = h1 @ Wg2 (row-major out) ---
                ps_n2 = psum.tile([128, 2 * R], F32, tag="ps3")
                for k in range(2):
                    nc.tensor.matmul(ps_n2[:], h1[:, k * R:(k + 1) * R],
                                     w['Wg2'][:, k, :],
                                     start=(k == 0), stop=(k == 1))
                n2_loc = state.tile([128, 2 * R], BF, tag="n2_loc")
                nc.vector.tensor_copy(n2_loc[:], ps_n2[:])

                # --- exchange 2 ---
                if remote:
                    exchange_remote(n2_loc, n2f, sem_dn, sem_ln, 'dn')
                else:
                    exchange_collective(n2_loc, n2f, "n")

                # --- GCN layer 2: h_g = A @ N2 + bg2, feature-major ---
                ps_g2 = psum.tile([128, 2 * R], F32, tag="ps4")
                if remote:
                    with tc.tile_critical():
                        nc.tensor.wait_ge(sem_dn, rem['dn'])
                        for m in range(2):
                            sl = ps_g2[:, m * R:(m + 1) * R]
                            for c in range(NCORES):
                                nc.tensor.matmul(
                                    sl, n2f[:, c, m * 128:(m + 1) * 128],
                                    adjt[:, c, :],
                                    start=(c == 0), stop=(c == NCORES - 1))
                else:
                    for m in range(2):
                        sl = ps_g2[:, m * R:(m + 1) * R]
                        for c in range(NCORES):
                            nc.tensor.matmul(
                                sl, n2f[:, c, m * 128:(m + 1) * 128],
                                adjt[:, c, :],
                                start=(c == 0), stop=(c == NCORES - 1))
                hg = work.tile([128, 2 * R], BF, tag="hg")
                for c in range(2):
                    nc.vector.tensor_scalar_add(
                        hg[:, c * R:(c + 1) * R], ps_g2[:, c * R:(c + 1) * R],
                        bsl('bg2', c))

                # --- h_next = [hg, h_new] @ Wlg + blg (feature-major) ---
                ps_h = psum.tile([128, 2 * R], F32, tag="ps3")
                cat = [hg[:, 0:R], hg[:, R:2 * R],
                       h_new[:, 0:R], h_new[:, R:2 * R]]
                for m in range(2):
                    sl = ps_h[:, m * R:(m + 1) * R]
                    for k in range(4):
                        nc.tensor.matmul(sl, w['Wlg'][:, k, m * 128:(m + 1) * 128],
                                         cat[k], start=(k == 0), stop=(k == 3))
                h_nxt = state.tile([128, 2 * R], BF, tag="h")
                for c in range(2):
                    nc.vector.tensor_scalar_add(
                        h_nxt[:, c * R:(c + 1) * R], ps_h[:, c * R:(c + 1) * R],
                        bsl('blg', c))
                h_prev = h_nxt

            # ---- outputs ----
            hf = work.tile([128, 2, R], F32, tag="hf")
            for c in range(2):
                nc.vector.tensor_copy(hf[:, c, :], h_prev[:, c * R:(c + 1) * R])
            nc.sync.dma_start(d_hout[:], hf[:])
            nc.sync.dma_start(d_kld[:], kldacc[:])
            nc.sync.dma_start(d_nll[:], nllacc[:])

    nc.compile()
    return nc


def build_probe():
    """Tiny kernel that discovers which logical rank lands in which slot of
    the remote-exchange buffer (the Q7 XORs *physical* TPB ids, and the
    logical->physical map is not ours to assume)."""
    nc = bacc.Bacc(
        "TRN2", target_bir_lowering=False, debug=False,
        enable_asserts=True, num_devices=NCORES,
    )
    d_myid = nc.dram_tensor("myid", [128, 16], F32, kind="ExternalInput")
    d_out = nc.dram_tensor("slots_out", [128, NCORES, 16], F32,
                           kind="ExternalOutput")
    sem_d = nc.alloc_semaphore("sem_d")
    sem_l = nc.alloc_semaphore("sem_l")
    rg = [list(range(NCORES))]
    with tile.TileContext(nc) as tc:
        with (
            tc.tile_pool(name="const", bufs=1) as const,
            tc.tile_pool(name="dram", bufs=2, space="DRAM") as dram,
        ):
            myid = const.tile([128, 16], BF, tag="myid")
            slots = const.tile([128, NCORES, 16], BF, tag="slots")
            bar_sb = const.tile([128, 16], BF, tag="bar_sb")
            with tc.tile_critical():
                nc.gpsimd.sem_clear(sem_d)
                nc.gpsimd.sem_clear(sem_l)
                nc.gpsimd.memset(bar_sb[:], 0.0)
            bar_in = dram.tile([128, 16], BF, tag="bar_in")
            bar_out = dram.tile([NCORES * 128, 16], BF, tag="bar_out")
            nc.sync.dma_start(bar_in[:], bar_sb[:])
            nc.gpsimd.collective_compute(
                "AllGather", ALU.bypass, ins=[bar_in.opt()],
                outs=[bar_out.opt()], replica_groups=rg)
            bar_back = const.tile([128, 16], BF, tag="bar_back")
            nc.sync.dma_start(bar_back[:], bar_out[:].rearrange(
                "(c p) n -> p c n", p=128)[:, 0, :])
            # myid load must come after the barrier readback so sends are late
            myid_f = const.tile([128, 16], F32, tag="myid_f")
            nc.sync.dma_start(myid_f[:], d_myid[:])
            nc.vector.tensor_add(myid[:], myid_f[:], bar_back[:])
            for d in range(1, NCORES):
                rdests = [None] * NCORES
                rdests[d] = (0, d)
                nc.gpsimd.remote_dma_broadcast(
                    slots[:, d, :], myid[:], sem_d, sem_l, rdests=rdests)
            nc.gpsimd.trigger_dma(count=None)
            nc.vector.tensor_copy(slots[:, 0, :], myid[:])
            outf = const.tile([128, NCORES, 16], F32, tag="outf")
            with tc.tile_critical():
                nc.vector.wait_ge(sem_d, 14)
                nc.vector.tensor_copy(outf[:], slots[:])
            nc.sync.dma_start(d_out[:], outf[:])
    nc.compile()
    return nc


_PROBE_CACHE = {}


def probe_slot_perms(sim=False):
    """Returns perms[r][d] = logical rank whose shard lands in slot d on
    logical core r."""
    key = 'sim' if sim else 'hw'
    if key in _PROBE_CACHE:
        return _PROBE_CACHE[key]
    nc = build_probe()
    in_maps = [{'myid': np.full((128, 16), float(j), np.float32)}
               for j in range(NCORES)]
    if sim:
        from concourse.bass_interp import MultiCoreSim
        s = MultiCoreSim(nc, num_cores=NCORES, trace=False)
        for c in range(NCORES):
            s.cores[c].tensor('myid')[:] = in_maps[c]['myid']
        s.simulate(check_with_hw=False)
        outs = [np.array(s.cores[c].tensor('slots_out')) for c in range(NCORES)]
    else:
        res = run_bass_kernel_spmd(nc, in_maps, list(range(NCORES)))
        outs = [np.asarray(res.results[c]['slots_out']) for c in range(NCORES)]
    perms = []
    for r in range(NCORES):
        pr = [int(round(float(outs[r][0, d, 0]))) for d in range(NCORES)]
        if sorted(pr) != list(range(NCORES)):
            raise RuntimeError(f"probe failed on core {r}: slots={pr}")
        perms.append(pr)
    _PROBE_CACHE[key] = perms
    return perms


# ------------------------------------------------------------- host side

def _prep_inputs(x, adj, eps, params, nsteps=NSTEPS, exchange_mode=EXCHANGE_MODE,
                 slot_perms=None):
    """Build the 8 per-core input maps from the full inputs."""
    bf = ml_dtypes.bfloat16
    p = {k: np.asarray(v, np.float32) for k, v in params.items()}
    x = np.asarray(x, np.float32)
    adj = np.asarray(adj, np.float32)
    eps = np.asarray(eps, np.float32)

    weights = {
        'Wpx1': p['Wpx1'], 'Wpx2': p['Wpx2'],
        'Wpz1': p['Wpz1'], 'Wpz2': p['Wpz2'],
        'We1': p['We1'], 'We2': p['We2'],
        'Wmlv_e': np.concatenate([p['Wem'], p['Wel']], 1),
        'Wp1': p['Wp1'], 'Wp2': p['Wp2'],
        'Wmlv_p': np.concatenate([p['Wpm'], p['Wpl']], 1),
        'Wd1': p['Wd1'], 'Wd2': p['Wd2'],
        'Wmlv_d': np.concatenate(
            [p['Wdm'], np.zeros((HD, 30), np.float32), p['Wdl']], 1),
        'Wih': p['Wih'], 'Whh': p['Whh'],
        'Wg1': p['Wg1'], 'Wg2': p['Wg2'], 'Wlg': p['Wlg'],
    }
    biases = {
        'bpx1': p['bpx1'], 'bpx2': p['bpx2'],
        'bpz1': p['bpz1'], 'bpz2': p['bpz2'],
        'be1': p['be1'], 'be2': p['be2'],
        'bmlv_e': np.concatenate([p['bem'], p['bel']]),
        'bp1': p['bp1'], 'bp2': p['bp2'],
        'bmlv_p': np.concatenate([p['bpm'], p['bpl']]),
        'bd1': p['bd1'], 'bd2': p['bd2'],
        'bmlv_d': np.concatenate(
            [p['bdm'], np.zeros(30, np.float32), p['bdl']]),
        'brz': 0.5 * (p['bih'][:512] + p['bhh'][:512]),
        'bihn': p['bih'][512:], 'bhhn': p['bhh'][512:],
        'bg1': p['bg1'], 'bg2': p['bg2'], 'blg': p['blg'],
    }
    wmaps = {k: np.ascontiguousarray(v).astype(bf) for k, v in weights.items()}
    bmaps = {k: np.ascontiguousarray(v, np.float32) for k, v in biases.items()}

    a = adj[1:1 + nsteps]  # (nsteps, B, B)
    in_maps = []
    for j in range(NCORES):
        rows = slice(j * R, (j + 1) * R)
        xj = x[1:1 + nsteps, rows, :].transpose(0, 2, 1)       # (t, XD, R)
        xj = np.ascontiguousarray(
            xj.transpose(1, 0, 2).reshape(XD, nsteps * R), np.float32)
        ej = eps[:nsteps, rows, :].transpose(0, 2, 1)
        ej = np.ascontiguousarray(
            ej.transpose(1, 0, 2).reshape(ZD, nsteps * R), np.float32)
        # adjacency slot layout: adjt[t, p, d, r] = adj[t+1, row_j+r, chunk(d)*128+p]
        aj = a[:, rows, :].reshape(nsteps, R, NCORES, 128)     # [t, r, c, p]
        if exchange_mode == "remote":
            perm = slot_perms[j] if slot_perms else [j ^ d for d in range(NCORES)]
            aj = aj[:, :, perm, :]
        adjt = np.ascontiguousarray(aj.transpose(0, 3, 2, 1)).astype(bf)
        im = {'xbf': xj.astype(bf), 'xf': xj, 'eps': ej, 'adjt': adjt}
        im.update(wmaps)
        im.update(bmaps)
        in_maps.append(im)
    return in_maps


def _assemble(results, nsteps=NSTEPS):
    KLD_sum = 0.0
    NLL_sum = 0.0
    h_full = np.zeros((B, RD), np.float32)
    for j in range(NCORES):
        r = results[j]
        KLD_sum += float(np.asarray(r['kldacc'], np.float64).sum())
        NLL_sum += float(np.asarray(r['nllacc'], np.float64).sum())
        hout = np.asarray(r['hout'], np.float32)      # (128, 2, R): [p, c, r]
        hj = hout.transpose(2, 1, 0).reshape(R, RD)   # [r, c*128+p]
        h_full[j * R:(j + 1) * R] = hj
    KLD = 0.5 * (KLD_sum / B) - 0.5 * ZD * nsteps
    NLL = 0.5 * (NLL_sum / B) + nsteps * LOG2PI
    return (np.array([KLD], np.float32), np.array([NLL], np.float32),
            h_full[None].astype(np.float32))


_NC_CACHE = {}


def kernel(x, adj, eps, params):
    mode = EXCHANGE_MODE
    perms = None
    if mode == "remote":
        try:
            perms = probe_slot_perms(sim=False)
        except Exception as e:
            print(f"kernel: remote-exchange probe failed ({e}); "
                  f"falling back to collective mode")
            mode = "collective"
    key = (NSTEPS, mode)
    if key not in _NC_CACHE:
        _NC_CACHE[key] = build_kernel(NSTEPS, mode)
    nc = _NC_CACHE[key]
    in_maps = _prep_inputs(x, adj, eps, params, NSTEPS, mode, perms)
    res = run_bass_kernel_spmd(nc, in_maps, list(range(NCORES)))
    return _assemble(res.results, NSTEPS)
